# revision 28
# baseline (speedup 1.0000x reference)
"""Trainium2 Bass kernel for nn_Decoder (attention-LSTM decoder recurrence).

Math (per batch b, T=128 steps, M=P=64):
    repeat t = 0..T-2:
        e = tanh(H @ U_d.T + W_d @ [h; c])          (T, M)
        s = exp(v_d . e)                            (T,)   softmax numerator
        num = sum_t s_t * (H w~[1:] + w~b + w~0 dec_t)_t
        den = sum_t s_t
        y~  = num / den                             (dec folded into num)
        LSTM(y~, h, c) -> h, c                      (i,f,g,o gates)
    final: attend once more; out = [h, ctx]

Sharding: data-parallel over batch. B=32 over 8 cores -> 4 batches/core.

The recurrence is latency-bound (fixed per-instruction engine latencies),
so the step loop is structured to minimize serial engine stages:
  - e_pre = UH + W q is accumulated in PSUM by matmuls (UH re-materialized
    each step from f16 H, off the critical path; q contribution uses
    stride-0 broadcast rhs), so tanh(e) is ONE bias-free ACT instr.
  - hw_all = H w~[1:] + w~b + w~0 dec[b,t] is host-precomputed, folding
    the y~ add into the num matmul (y~ = num/den, one DVE divide).
  - den matmul issues before the num matmuls.
  - gates = Whh [2h; 1] matmuls issue early (hidden behind attention);
    only the rank-1 wih (x) y~ accumulation is on the critical path.
  - sV = (ti+1)*tg runs on gpsimd in parallel with DVE's sU.
State stores 2h / 2c (sigmoid(z) = 0.5 tanh(0.5 z) + 0.5 folding; the 0.5s
live in the host-packed weights).
"""

import numpy as np

B, T, M, P = 32, 128, 64, 64
NCORES = 8
BL = B // NCORES          # batches per core = 4
NG = 2                    # attention groups per core
GB = BL // NG             # batches per attention group = 2

_STATE = {}
NSTEPS = T - 1
DEBUG = False
PSTATE_PAD = True     # keep PE continuously busy so it ramps to full clock


def _build_nc():
    import contextlib

    import concourse.bacc as bacc
    import concourse.tile as tile
    from concourse import mybir

    f32 = mybir.dt.float32
    f32r = mybir.dt.float32r
    f16 = mybir.dt.float16
    AF = mybir.ActivationFunctionType
    OP = mybir.AluOpType

    nc = bacc.Bacc()

    # ---- per-core sharded data ----
    h_l = nc.declare_dram_parameter("h_l", [BL, T, M], f32, isOutput=False)
    htp = nc.declare_dram_parameter("htp", [GB, M, NG * T], f16, isOutput=False)
    hw_all = nc.declare_dram_parameter("hw_all", [T, 4 * T], f16, isOutput=False)
    st0 = nc.declare_dram_parameter("st0", [2 * P, BL], f16, isOutput=False)
    # ---- replicated packed weights ----
    wd2 = nc.declare_dram_parameter("wd2", [P, 2 * M], f16, isOutput=False)
    udT16 = nc.declare_dram_parameter("udT16", [M, M], f16, isOutput=False)
    v2 = nc.declare_dram_parameter("v2", [2 * M, GB], f16, isOutput=False)
    whbi = nc.declare_dram_parameter("whbi", [P + 2, 4 * P], f16, isOutput=False)
    # ---- outputs ----
    oh = nc.declare_dram_parameter("oh", [P, BL], f32, isOutput=True)
    octx = nc.declare_dram_parameter("octx", [M + 1, BL], f32, isOutput=True)
    if DEBUG:
        dbg_g = nc.declare_dram_parameter("dbg_g", [P, 4 * BL], f32, isOutput=True)

    with tile.TileContext(nc) as tc:
        with contextlib.ExitStack() as ctx:
            consts = ctx.enter_context(tc.tile_pool(name="consts", bufs=1))
            state = ctx.enter_context(tc.tile_pool(name="state", bufs=1))
            loop_sb = ctx.enter_context(tc.tile_pool(name="loop_sb", bufs=3))
            loop_ps = ctx.enter_context(
                tc.tile_pool(name="loop_ps", bufs=1, space="PSUM")
            )
            ep_pool = ctx.enter_context(
                tc.tile_pool(name="ep_ps", bufs=2, space="PSUM")
            )

            # ---------------- preamble: constants ----------------
            wd2_sb = consts.tile([P, 2 * M], f16)
            nc.sync.dma_start(out=wd2_sb, in_=wd2[:])
            udT_sb = consts.tile([M, M], f16)
            nc.sync.dma_start(out=udT_sb, in_=udT16[:])
            v2_sb = consts.tile([2 * M, GB], f16)
            nc.sync.dma_start(out=v2_sb, in_=v2[:])
            whbi_sb = consts.tile([P + 2, 4 * P], f16)
            nc.sync.dma_start(out=whbi_sb, in_=whbi[:])
            hw_sb = consts.tile([T, 4 * T], f16)
            nc.sync.dma_start(out=hw_sb, in_=hw_all[:])
            htp_sb = []
            for j in range(GB):
                t_ = consts.tile([M, NG * T], f16, tag=f"HTP{j}")
                nc.sync.dma_start(out=t_, in_=htp[j])
                htp_sb.append(t_)
            ones_f = consts.tile([T, 1], f32)
            nc.vector.memset(ones_f, 1.0)
            ones16 = consts.tile([T, 1], f16)
            nc.vector.tensor_copy(out=ones16, in_=ones_f)

            # state: sy2 = [2h (0:64); y~ (64); 1 (65)], cs = 2c
            sy2 = state.tile([P + 2, BL], f16, tag="SY")
            ones_g = state.tile([P + 2, BL], f32, tag="ONESG")
            nc.vector.memset(ones_g[P:P + 2, :], 1.0)
            nc.vector.tensor_copy(out=sy2[P:P + 2, :], in_=ones_g[P:P + 2, :])
            nc.sync.dma_start(out=sy2[0:P, :], in_=st0[0:P, :])
            cs = state.tile([P, BL], f16, tag="CS")
            nc.sync.dma_start(out=cs, in_=st0[P:2 * P, :])

            haug = []
            for b in range(BL):
                hb = consts.tile([T, M], f32r, tag=f"HAUG{b}")
                nc.sync.dma_start(out=hb, in_=h_l[b].bitcast(f32r))
                haug.append(hb)

            # scratch bank for p-state padding matmuls (write-only)
            if PSTATE_PAD:
                pad_ps = ctx.enter_context(
                    tc.tile_pool(name="pad_ps", bufs=1, space="PSUM"))

            def pad(n):
                # dummy matmuls with no waits: they fill PE idle windows so
                # the tensor engine stays busy and ramps to full clock
                if not PSTATE_PAD:
                    return
                for _ in range(n):
                    dtile = pad_ps.tile([M, NG * T], f32, tag="PAD")
                    nc.tensor.matmul(dtile, udT_sb, htp_sb[0],
                                     start=True, stop=True)

            # ---------- one step's attention front: e_pre..num/den ----------
            def attention(t, need_num=True):
                ep = ep_pool.tile([2 * M, NG * T], f32, tag="EP")
                epr = ep.rearrange("p (g t) -> p g t", g=NG)
                # UH accumulation (consts only -> runs during prev LSTM tail)
                for j in range(GB):
                    nc.tensor.matmul(
                        ep[j * M:(j + 1) * M, :], udT_sb, htp_sb[j],
                        start=True, stop=False)
                # qW c-half then h-half, broadcast over t
                csr = cs.rearrange("p (g j) -> p g j", j=GB)
                for j in range(GB):
                    nc.tensor.matmul(
                        epr[j * M:(j + 1) * M, :, :], wd2_sb[:, 0:M],
                        csr[:, :, j].to_broadcast([P, NG, T]),
                        start=False, stop=False, skip_group_check=True)
                pad(3)
                syr = sy2[0:P, :].rearrange("p (g j) -> p g j", j=GB)
                for j in range(GB):
                    nc.tensor.matmul(
                        epr[j * M:(j + 1) * M, :, :], wd2_sb[:, M:2 * M],
                        syr[:, :, j].to_broadcast([P, NG, T]),
                        start=False, stop=True, skip_group_check=True)
                pad(1)
                # e = tanh(e_pre): single bias-free ACT instr
                e_sb = loop_sb.tile([2 * M, NG * T], f16, tag="e")
                nc.scalar.activation(out=e_sb, in_=ep, func=AF.Tanh)
                # logits
                lg = loop_ps.tile([T, 2 * GB], f32, tag="LG")
                for g in range(NG):
                    nc.tensor.matmul(
                        lg[:, g * GB:(g + 1) * GB],
                        e_sb[:, g * T:(g + 1) * T], v2_sb,
                        start=True, stop=True)
                # softmax numerators
                pad(1)
                s_sb = loop_sb.tile([T, BL], f16, tag="s")
                nc.scalar.activation(out=s_sb, in_=lg, func=AF.Exp)
                # den first (its reciprocal overlaps the num matmuls);
                # scalar row-world lives at partition 64 (aligns with the
                # y~ slot in sy2 for the DVE ops)
                dn = loop_ps.tile([P + 2, 2 * BL], f32, tag="DN")
                nc.tensor.matmul(dn[P:P + 1, 0:BL], ones16, s_sb,
                                 start=True, stop=True)
                if need_num:
                    for b in range(BL):
                        nc.tensor.matmul(
                            dn[P:P + 1, BL + b:BL + b + 1],
                            hw_sb[:, 4 * t + b:4 * t + b + 1],
                            s_sb[:, b:b + 1], start=True, stop=True)
                pad(1)
                return dn, s_sb

            # ---------------- main recurrence ----------------
            for t in range(NSTEPS):
                dn, _ = attention(t)
                # y~ = num * (1/den); recip issues right after the den
                # matmul and overlaps the num matmuls; y~ lands in sy2
                rden = loop_sb.tile([P + 2, BL], f32, tag="rden")
                nc.vector.reciprocal(
                    out=rden[P:P + 1, :], in_=dn[P:P + 1, 0:BL])
                nc.vector.tensor_tensor(
                    out=sy2[P:P + 1, :], in0=dn[P:P + 1, BL:2 * BL],
                    in1=rden[P:P + 1, :], op=OP.mult)
                # gates: single complete group, lhsT = [Whh; wih; b]
                gps = loop_ps.tile([P, 4 * BL], f32, tag="G")
                for k in range(4):
                    nc.tensor.matmul(
                        gps[:, k * BL:(k + 1) * BL],
                        whbi_sb[:, k * P:(k + 1) * P], sy2,
                        start=True, stop=True)
                tg = loop_sb.tile([P, 4 * BL], f32, tag="T")
                nc.scalar.activation(out=tg, in_=gps, func=AF.Tanh)
                if DEBUG and t == 0:
                    g32 = loop_sb.tile([P, 4 * BL], f32, tag="g32")
                    nc.vector.tensor_copy(out=g32, in_=gps)
                    nc.sync.dma_start(out=dbg_g[:], in_=g32)
                t_i, t_f = tg[:, 0:BL], tg[:, BL:2 * BL]
                t_o, t_g = tg[:, 2 * BL:3 * BL], tg[:, 3 * BL:4 * BL]
                # CS holds Xc = 2c; X = 0.5*(tf+1)*Xc + (ti+1)*tg = 2c_new
                sU = loop_sb.tile([P, BL], f32, tag="sU")
                nc.vector.scalar_tensor_tensor(
                    out=sU, in0=t_f, scalar=1.0, in1=cs,
                    op0=OP.add, op1=OP.mult)
                sV = loop_sb.tile([P, BL], f32, tag="sV")
                nc.vector.scalar_tensor_tensor(
                    out=sV, in0=t_i, scalar=1.0, in1=t_g,
                    op0=OP.add, op1=OP.mult)
                nc.vector.scalar_tensor_tensor(
                    out=cs, in0=sU, in1=sV, scalar=0.5,
                    op0=OP.mult, op1=OP.add)
                th = loop_sb.tile([P, BL], f32, tag="th")
                nc.scalar.activation(out=th, in_=cs, func=AF.Tanh, scale=0.5)
                # 2h_new = (to+1)*th
                nc.vector.scalar_tensor_tensor(
                    out=sy2[0:P, :], in0=t_o, scalar=1.0, in1=th,
                    op0=OP.add, op1=OP.mult)

            # ---------------- final attend + outputs ----------------
            dn, s_fin = attention(None, need_num=False)
            s_fr = loop_sb.tile([T, BL], f32r, tag="sfr")
            nc.vector.tensor_copy(out=s_fr, in_=s_fin)
            ctx_ps = loop_ps.tile([M, 2 * BL], f32, tag="CTXF")
            for b in range(BL):
                nc.tensor.matmul(
                    ctx_ps[:, 2 * b:2 * b + 2], haug[b],
                    s_fr[:, b:b + 1].to_broadcast([T, 2]),
                    start=True, stop=True)
            ctx_out = loop_sb.tile([M, BL], f32, tag="ctxout")
            nc.vector.tensor_copy(
                out=ctx_out,
                in_=ctx_ps.rearrange("p (b two) -> p b two", two=2)[:, :, 0])
            den_out = loop_sb.tile([P + 2, BL], f32, tag="denout")
            nc.vector.tensor_copy(
                out=den_out[P:P + 1, :], in_=dn[P:P + 1, 0:BL])
            oh_sb = loop_sb.tile([P, BL], f32, tag="ohsb")
            nc.vector.tensor_copy(out=oh_sb, in_=sy2[0:P, :])
            nc.sync.dma_start(out=octx[0:M, :], in_=ctx_out)
            nc.sync.dma_start(out=octx[M:M + 1, :], in_=den_out[P:P + 1, :])
            nc.gpsimd.dma_start(out=oh[:], in_=oh_sb)

    nc.finalize()
    return nc


def _pack_weights(W_d, U_d, v_d, w_tilde_W, w_tilde_b, W_ih, W_hh, b_ih, b_hh):
    f16 = np.float16
    # q = [h;c] stored as 2h;2c -> fold 0.5 into W_d^T halves
    # wd2 cols 0:M = c-half lhsT, cols M:2M = h-half lhsT
    wd2 = np.zeros((P, 2 * M), dtype=f16)
    wd2[:, 0:M] = 0.5 * W_d[:, P:2 * P].T
    wd2[:, M:2 * M] = 0.5 * W_d[:, 0:P].T
    udT16 = np.ascontiguousarray(U_d.T, dtype=f16)
    v2 = np.zeros((2 * M, GB), dtype=f16)
    v2[0:M, 0] = v_d[0]
    v2[M:2 * M, 1] = v_d[0]
    bsum = (b_ih + b_hh).astype(np.float32)
    wih = W_ih[:, 0].astype(np.float32)
    # torch gate order i,f,g,o; our column order i,f,o,g.
    # sigmoid gates (i,f,o): pre-scale 0.5 (sigmoid(z) = 0.5 tanh(0.5 z)+0.5)
    # h input is 2h -> extra 0.5 on W_hh blocks.
    src = [0, 1, 3, 2]                    # i, f, o, g row-blocks in torch order
    sig = [0.5, 0.5, 0.5, 1.0]
    whbi = np.zeros((P + 2, 4 * P), dtype=f16)
    for k in range(4):
        blk = slice(src[k] * P, (src[k] + 1) * P)
        whbi[0:P, k * P:(k + 1) * P] = sig[k] * 0.5 * W_hh[blk].T
        whbi[P, k * P:(k + 1) * P] = sig[k] * wih[blk]
        whbi[P + 1, k * P:(k + 1) * P] = sig[k] * bsum[blk]
    return dict(wd2=wd2, udT16=udT16, v2=v2, whbi=whbi)


def kernel(H, dec_data, d_1, s_1, W_d, U_d, v_d, w_tilde_W, w_tilde_b,
           W_ih, W_hh, b_ih, b_hh, T=None):
    from concourse.bass_utils import run_bass_kernel_spmd

    H = np.asarray(H, dtype=np.float32)
    dec_data = np.asarray(dec_data, dtype=np.float32)
    d_1 = np.asarray(d_1, dtype=np.float32)
    s_1 = np.asarray(s_1, dtype=np.float32)
    W_d = np.asarray(W_d, np.float32)
    w_tilde_W = np.asarray(w_tilde_W, np.float32)
    w_tilde_b = np.asarray(w_tilde_b, np.float32)

    if "nc" not in _STATE:
        _STATE["nc"] = _build_nc()
    nc = _STATE["nc"]

    wpack = _pack_weights(
        W_d, np.asarray(U_d, np.float32),
        np.asarray(v_d, np.float32), w_tilde_W, w_tilde_b,
        np.asarray(W_ih, np.float32), np.asarray(W_hh, np.float32),
        np.asarray(b_ih, np.float32), np.asarray(b_hh, np.float32),
    )

    wt1 = w_tilde_W[0, 1:M + 1]                         # [64]
    w0 = w_tilde_W[0, 0]
    wtb = w_tilde_b[0]

    in_maps = []
    for core in range(NCORES):
        sl = slice(core * BL, (core + 1) * BL)
        h_c = H[sl]                                     # [4, T, M]
        # htp[j][m, g*T + t] = H[2g+j][t, m]
        htp = np.zeros((GB, M, NG * 128), dtype=np.float16)
        for j in range(GB):
            for g in range(NG):
                htp[j, :, g * 128:(g + 1) * 128] = h_c[NG * g + j].T
        # hw_all[t, 4*s + b] = (H_b @ wt1)[t] + wtb + w0*dec[b, s]
        hwb = h_c @ wt1 + wtb                           # [4, T]
        dec_c = dec_data[sl, :, 0]                      # [4, T]
        # X[s, b, t] = hwb[b, t] + w0*dec[b, s]; want hw_all[t, 4*s+b]
        hw_all = (hwb[None, :, :] + (w0 * dec_c).T[:, :, None]
                  ).transpose(2, 0, 1).reshape(128, 4 * 128)
        st = np.concatenate(
            [2.0 * d_1[0, sl].T, 2.0 * s_1[0, sl].T], axis=0
        ).astype(np.float16)                            # [128, 4]
        m = dict(wpack)
        m.update(
            h_l=np.ascontiguousarray(h_c),
            htp=htp,
            hw_all=np.ascontiguousarray(hw_all.astype(np.float16)),
            st0=np.ascontiguousarray(st),
        )
        in_maps.append(m)

    res = run_bass_kernel_spmd(nc, in_maps, list(range(NCORES)))
    _STATE["last_results"] = res

    out = np.zeros((B, 1, P + M), dtype=np.float32)
    for core in range(NCORES):
        r = res.results[core]
        hv = r["oh"].T * 0.5                      # [4, 64]  (state was 2h)
        octx = r["octx"]
        ctxv = (octx[0:M] / octx[M:M + 1]).T      # [4, 64]
        out[core * BL:(core + 1) * BL, 0, 0:P] = hv
        out[core * BL:(core + 1) * BL, 0, P:P + M] = ctxv
    return out


# revision 34
# speedup vs baseline: 1.2165x; 1.2165x over previous
"""Trainium2 Bass kernel for nn_Decoder (attention-LSTM decoder recurrence).

Math (per batch b, T=128 steps, M=P=64):
    repeat t = 0..T-2:
        e = tanh(H @ U_d.T + W_d @ [h; c])          (T, M)
        s = exp(v_d . e)                            (T,)   softmax numerator
        num = sum_t s_t * (H w~[1:] + w~b + w~0 dec_t)_t
        den = sum_t s_t
        y~  = num / den                             (dec folded into num)
        LSTM(y~, h, c) -> h, c                      (i,f,g,o gates)
    final: attend once more; out = [h, ctx]

Sharding: data-parallel over batch. B=32 over 8 cores -> 4 batches/core.

The recurrence is latency-bound (fixed per-instruction engine latencies),
so the step loop is structured to minimize serial engine stages:
  - e_pre = UH + W q is accumulated in PSUM by matmuls (UH re-materialized
    each step from f16 H, off the critical path; q contribution uses
    stride-0 broadcast rhs), so tanh(e) is ONE bias-free ACT instr.
  - hw_all = H w~[1:] + w~b + w~0 dec[b,t] is host-precomputed, folding
    the y~ add into the num matmul (y~ = num/den, one DVE divide).
  - den matmul issues before the num matmuls.
  - gates = Whh [2h; 1] matmuls issue early (hidden behind attention);
    only the rank-1 wih (x) y~ accumulation is on the critical path.
  - sV = (ti+1)*tg runs on gpsimd in parallel with DVE's sU.
State stores 2h / 2c (sigmoid(z) = 0.5 tanh(0.5 z) + 0.5 folding; the 0.5s
live in the host-packed weights).
"""

import numpy as np

B, T, M, P = 32, 128, 64, 64
NCORES = 8
BL = B // NCORES          # batches per core = 4
NG = 2                    # attention groups per core
GB = BL // NG             # batches per attention group = 2

_STATE = {}
NSTEPS = T - 1
DEBUG = False
PSTATE_PAD = False    # PE p-state padding: measured no clock ramp on HW


def _build_nc():
    import contextlib

    import concourse.bacc as bacc
    import concourse.tile as tile
    from concourse import mybir

    f32 = mybir.dt.float32
    f32r = mybir.dt.float32r
    f16 = mybir.dt.float16
    AF = mybir.ActivationFunctionType
    OP = mybir.AluOpType

    nc = bacc.Bacc()

    # ---- per-core sharded data ----
    h_l = nc.declare_dram_parameter("h_l", [BL, T, M], f32, isOutput=False)
    htp = nc.declare_dram_parameter("htp", [GB, M, NG * T], f16, isOutput=False)
    hw_all = nc.declare_dram_parameter("hw_all", [T, 4 * T], f16, isOutput=False)
    st0 = nc.declare_dram_parameter("st0", [2 * P, BL], f16, isOutput=False)
    # ---- replicated packed weights ----
    wd2 = nc.declare_dram_parameter("wd2", [P, 2 * M], f16, isOutput=False)
    udT16 = nc.declare_dram_parameter("udT16", [M, M], f16, isOutput=False)
    v2 = nc.declare_dram_parameter("v2", [2 * M, GB], f16, isOutput=False)
    whbi = nc.declare_dram_parameter("whbi", [P + 2, 4 * P], f16, isOutput=False)
    # ---- outputs ----
    oh = nc.declare_dram_parameter("oh", [P, BL], f32, isOutput=True)
    octx = nc.declare_dram_parameter("octx", [M + 1, BL], f32, isOutput=True)
    if DEBUG:
        dbg_g = nc.declare_dram_parameter("dbg_g", [P, 4 * BL], f32, isOutput=True)

    with tile.TileContext(nc) as tc:
        with contextlib.ExitStack() as ctx:
            consts = ctx.enter_context(tc.tile_pool(name="consts", bufs=1))
            state = ctx.enter_context(tc.tile_pool(name="state", bufs=1))
            loop_sb = ctx.enter_context(tc.tile_pool(name="loop_sb", bufs=3))
            loop_ps = ctx.enter_context(
                tc.tile_pool(name="loop_ps", bufs=1, space="PSUM")
            )
            ep_pool = ctx.enter_context(
                tc.tile_pool(name="ep_ps", bufs=2, space="PSUM")
            )

            # ---------------- preamble: constants ----------------
            wd2_sb = consts.tile([P, 2 * M], f16)
            nc.sync.dma_start(out=wd2_sb, in_=wd2[:])
            udT_sb = consts.tile([M, M], f16)
            nc.sync.dma_start(out=udT_sb, in_=udT16[:])
            v2_sb = consts.tile([2 * M, GB], f16)
            nc.sync.dma_start(out=v2_sb, in_=v2[:])
            whbi_sb = consts.tile([P + 2, 4 * P], f16)
            nc.sync.dma_start(out=whbi_sb, in_=whbi[:])
            hw_sb = consts.tile([T, 4 * T], f16)
            nc.sync.dma_start(out=hw_sb, in_=hw_all[:])
            htp_sb = []
            for j in range(GB):
                t_ = consts.tile([M, NG * T], f16, tag=f"HTP{j}")
                nc.sync.dma_start(out=t_, in_=htp[j])
                htp_sb.append(t_)
            ones_f = consts.tile([T, 1], f32)
            nc.vector.memset(ones_f, 1.0)
            ones16 = consts.tile([T, 1], f16)
            nc.vector.tensor_copy(out=ones16, in_=ones_f)

            # state: sy2 = [2h (0:64); y~ (64); 1 (65)], cs = 2c
            sy2 = state.tile([P + 2, BL], f16, tag="SY")
            ones_g = state.tile([P + 2, BL], f32, tag="ONESG")
            nc.vector.memset(ones_g[P:P + 2, :], 1.0)
            nc.vector.tensor_copy(out=sy2[P:P + 2, :], in_=ones_g[P:P + 2, :])
            nc.sync.dma_start(out=sy2[0:P, :], in_=st0[0:P, :])
            cs = state.tile([P, BL], f16, tag="CS")
            nc.sync.dma_start(out=cs, in_=st0[P:2 * P, :])

            haug = []
            for b in range(BL):
                hb = consts.tile([T, M], f32r, tag=f"HAUG{b}")
                nc.sync.dma_start(out=hb, in_=h_l[b].bitcast(f32r))
                haug.append(hb)

            # scratch bank for p-state padding matmuls (write-only)
            if PSTATE_PAD:
                pad_ps = ctx.enter_context(
                    tc.tile_pool(name="pad_ps", bufs=1, space="PSUM"))

            def pad(n):
                # dummy matmuls with no waits: they fill PE idle windows so
                # the tensor engine stays busy and ramps to full clock
                if not PSTATE_PAD:
                    return
                for _ in range(n):
                    dtile = pad_ps.tile([M, NG * T], f32, tag="PAD")
                    nc.tensor.matmul(dtile, udT_sb, htp_sb[0],
                                     start=True, stop=True)

            # ---------- one step's attention front: e_pre..num/den ----------
            def attention(t, need_num=True):
                ep = ep_pool.tile([2 * M, NG * T], f32, tag="EP")
                epr = ep.rearrange("p (g t) -> p g t", g=NG)
                # UH accumulation (consts only -> runs during prev LSTM tail)
                for j in range(GB):
                    nc.tensor.matmul(
                        ep[j * M:(j + 1) * M, :], udT_sb, htp_sb[j],
                        start=True, stop=False)
                # qW c-half then h-half, broadcast over t
                csr = cs.rearrange("p (g j) -> p g j", j=GB)
                for j in range(GB):
                    nc.tensor.matmul(
                        epr[j * M:(j + 1) * M, :, :], wd2_sb[:, 0:M],
                        csr[:, :, j].to_broadcast([P, NG, T]),
                        start=False, stop=False, skip_group_check=True)
                syr = sy2[0:P, :].rearrange("p (g j) -> p g j", j=GB)
                for j in range(GB):
                    nc.tensor.matmul(
                        epr[j * M:(j + 1) * M, :, :], wd2_sb[:, M:2 * M],
                        syr[:, :, j].to_broadcast([P, NG, T]),
                        start=False, stop=True, skip_group_check=True)
                # e = tanh(e_pre): single bias-free ACT instr
                e_sb = loop_sb.tile([2 * M, NG * T], f16, tag="e")
                nc.scalar.activation(out=e_sb, in_=ep, func=AF.Tanh)
                # logits
                lg = loop_ps.tile([T, 2 * GB], f32, tag="LG")
                for g in range(NG):
                    nc.tensor.matmul(
                        lg[:, g * GB:(g + 1) * GB],
                        e_sb[:, g * T:(g + 1) * T], v2_sb,
                        start=True, stop=True)
                # softmax numerators
                s_sb = loop_sb.tile([T, BL], f16, tag="s")
                nc.scalar.activation(out=s_sb, in_=lg, func=AF.Exp)
                # den first (its reciprocal overlaps the num matmuls);
                # scalar row-world lives at partition 64 (aligns with the
                # y~ slot in sy2 for the DVE ops)
                dn = loop_ps.tile([P + 2, 2 * BL], f32, tag="DN")
                nc.tensor.matmul(dn[P:P + 1, 0:BL], ones16, s_sb,
                                 start=True, stop=True)
                if need_num:
                    for b in range(BL):
                        nc.tensor.matmul(
                            dn[P:P + 1, BL + b:BL + b + 1],
                            hw_sb[:, 4 * t + b:4 * t + b + 1],
                            s_sb[:, b:b + 1], start=True, stop=True)
                pad(1)
                return dn, s_sb

            # ---------------- main recurrence ----------------
            for t in range(NSTEPS):
                dn, _ = attention(t)
                # y~ = num * (1/den); recip issues right after the den
                # matmul and overlaps the num matmuls; y~ lands in sy2
                rden = loop_sb.tile([P + 2, BL], f32, tag="rden")
                nc.vector.reciprocal(
                    out=rden[P:P + 1, :], in_=dn[P:P + 1, 0:BL])
                nc.vector.tensor_tensor(
                    out=sy2[P:P + 1, :], in0=dn[P:P + 1, BL:2 * BL],
                    in1=rden[P:P + 1, :], op=OP.mult)
                # gates: single complete group, lhsT = [Whh; wih; b]
                gps = loop_ps.tile([P, 4 * BL], f32, tag="G")
                for k in range(4):
                    nc.tensor.matmul(
                        gps[:, k * BL:(k + 1) * BL],
                        whbi_sb[:, k * P:(k + 1) * P], sy2,
                        start=True, stop=True)
                tg = loop_sb.tile([P, 4 * BL], f32, tag="T")
                nc.scalar.activation(out=tg, in_=gps, func=AF.Tanh)
                if DEBUG and t == 0:
                    g32 = loop_sb.tile([P, 4 * BL], f32, tag="g32")
                    nc.vector.tensor_copy(out=g32, in_=gps)
                    nc.sync.dma_start(out=dbg_g[:], in_=g32)
                t_i, t_f = tg[:, 0:BL], tg[:, BL:2 * BL]
                t_o, t_g = tg[:, 2 * BL:3 * BL], tg[:, 3 * BL:4 * BL]
                # CS holds Xc = 2c; X = 0.5*(tf+1)*Xc + (ti+1)*tg = 2c_new
                sU = loop_sb.tile([P, BL], f32, tag="sU")
                nc.vector.scalar_tensor_tensor(
                    out=sU, in0=t_f, scalar=1.0, in1=cs,
                    op0=OP.add, op1=OP.mult)
                sV = loop_sb.tile([P, BL], f32, tag="sV")
                nc.vector.scalar_tensor_tensor(
                    out=sV, in0=t_i, scalar=1.0, in1=t_g,
                    op0=OP.add, op1=OP.mult)
                nc.vector.scalar_tensor_tensor(
                    out=cs, in0=sU, in1=sV, scalar=0.5,
                    op0=OP.mult, op1=OP.add)
                th = loop_sb.tile([P, BL], f32, tag="th")
                nc.scalar.activation(out=th, in_=cs, func=AF.Tanh, scale=0.5)
                # 2h_new = (to+1)*th
                nc.vector.scalar_tensor_tensor(
                    out=sy2[0:P, :], in0=t_o, scalar=1.0, in1=th,
                    op0=OP.add, op1=OP.mult)

            # ---------------- final attend + outputs ----------------
            dn, s_fin = attention(None, need_num=False)
            s_fr = loop_sb.tile([T, BL], f32r, tag="sfr")
            nc.vector.tensor_copy(out=s_fr, in_=s_fin)
            ctx_ps = loop_ps.tile([M, 2 * BL], f32, tag="CTXF")
            for b in range(BL):
                nc.tensor.matmul(
                    ctx_ps[:, 2 * b:2 * b + 2], haug[b],
                    s_fr[:, b:b + 1].to_broadcast([T, 2]),
                    start=True, stop=True)
            ctx_out = loop_sb.tile([M, BL], f32, tag="ctxout")
            nc.vector.tensor_copy(
                out=ctx_out,
                in_=ctx_ps.rearrange("p (b two) -> p b two", two=2)[:, :, 0])
            den_out = loop_sb.tile([P + 2, BL], f32, tag="denout")
            nc.vector.tensor_copy(
                out=den_out[P:P + 1, :], in_=dn[P:P + 1, 0:BL])
            oh_sb = loop_sb.tile([P, BL], f32, tag="ohsb")
            nc.vector.tensor_copy(out=oh_sb, in_=sy2[0:P, :])
            nc.sync.dma_start(out=octx[0:M, :], in_=ctx_out)
            nc.sync.dma_start(out=octx[M:M + 1, :], in_=den_out[P:P + 1, :])
            nc.gpsimd.dma_start(out=oh[:], in_=oh_sb)

    nc.finalize()
    return nc


def _pack_weights(W_d, U_d, v_d, w_tilde_W, w_tilde_b, W_ih, W_hh, b_ih, b_hh):
    f16 = np.float16
    # q = [h;c] stored as 2h;2c -> fold 0.5 into W_d^T halves
    # wd2 cols 0:M = c-half lhsT, cols M:2M = h-half lhsT
    wd2 = np.zeros((P, 2 * M), dtype=f16)
    wd2[:, 0:M] = 0.5 * W_d[:, P:2 * P].T
    wd2[:, M:2 * M] = 0.5 * W_d[:, 0:P].T
    udT16 = np.ascontiguousarray(U_d.T, dtype=f16)
    v2 = np.zeros((2 * M, GB), dtype=f16)
    v2[0:M, 0] = v_d[0]
    v2[M:2 * M, 1] = v_d[0]
    bsum = (b_ih + b_hh).astype(np.float32)
    wih = W_ih[:, 0].astype(np.float32)
    # torch gate order i,f,g,o; our column order i,f,o,g.
    # sigmoid gates (i,f,o): pre-scale 0.5 (sigmoid(z) = 0.5 tanh(0.5 z)+0.5)
    # h input is 2h -> extra 0.5 on W_hh blocks.
    src = [0, 1, 3, 2]                    # i, f, o, g row-blocks in torch order
    sig = [0.5, 0.5, 0.5, 1.0]
    whbi = np.zeros((P + 2, 4 * P), dtype=f16)
    for k in range(4):
        blk = slice(src[k] * P, (src[k] + 1) * P)
        whbi[0:P, k * P:(k + 1) * P] = sig[k] * 0.5 * W_hh[blk].T
        whbi[P, k * P:(k + 1) * P] = sig[k] * wih[blk]
        whbi[P + 1, k * P:(k + 1) * P] = sig[k] * bsum[blk]
    return dict(wd2=wd2, udT16=udT16, v2=v2, whbi=whbi)


def kernel(H, dec_data, d_1, s_1, W_d, U_d, v_d, w_tilde_W, w_tilde_b,
           W_ih, W_hh, b_ih, b_hh, T=None):
    from concourse.bass_utils import run_bass_kernel_spmd

    H = np.asarray(H, dtype=np.float32)
    dec_data = np.asarray(dec_data, dtype=np.float32)
    d_1 = np.asarray(d_1, dtype=np.float32)
    s_1 = np.asarray(s_1, dtype=np.float32)
    W_d = np.asarray(W_d, np.float32)
    w_tilde_W = np.asarray(w_tilde_W, np.float32)
    w_tilde_b = np.asarray(w_tilde_b, np.float32)

    if "nc" not in _STATE:
        _STATE["nc"] = _build_nc()
    nc = _STATE["nc"]

    wpack = _pack_weights(
        W_d, np.asarray(U_d, np.float32),
        np.asarray(v_d, np.float32), w_tilde_W, w_tilde_b,
        np.asarray(W_ih, np.float32), np.asarray(W_hh, np.float32),
        np.asarray(b_ih, np.float32), np.asarray(b_hh, np.float32),
    )

    wt1 = w_tilde_W[0, 1:M + 1]                         # [64]
    w0 = w_tilde_W[0, 0]
    wtb = w_tilde_b[0]

    in_maps = []
    for core in range(NCORES):
        sl = slice(core * BL, (core + 1) * BL)
        h_c = H[sl]                                     # [4, T, M]
        # htp[j][m, g*T + t] = H[2g+j][t, m]
        htp = np.zeros((GB, M, NG * 128), dtype=np.float16)
        for j in range(GB):
            for g in range(NG):
                htp[j, :, g * 128:(g + 1) * 128] = h_c[NG * g + j].T
        # hw_all[t, 4*s + b] = (H_b @ wt1)[t] + wtb + w0*dec[b, s]
        hwb = h_c @ wt1 + wtb                           # [4, T]
        dec_c = dec_data[sl, :, 0]                      # [4, T]
        # X[s, b, t] = hwb[b, t] + w0*dec[b, s]; want hw_all[t, 4*s+b]
        hw_all = (hwb[None, :, :] + (w0 * dec_c).T[:, :, None]
                  ).transpose(2, 0, 1).reshape(128, 4 * 128)
        st = np.concatenate(
            [2.0 * d_1[0, sl].T, 2.0 * s_1[0, sl].T], axis=0
        ).astype(np.float16)                            # [128, 4]
        m = dict(wpack)
        m.update(
            h_l=np.ascontiguousarray(h_c),
            htp=htp,
            hw_all=np.ascontiguousarray(hw_all.astype(np.float16)),
            st0=np.ascontiguousarray(st),
        )
        in_maps.append(m)

    res = run_bass_kernel_spmd(nc, in_maps, list(range(NCORES)))
    _STATE["last_results"] = res

    out = np.zeros((B, 1, P + M), dtype=np.float32)
    for core in range(NCORES):
        r = res.results[core]
        hv = r["oh"].T * 0.5                      # [4, 64]  (state was 2h)
        octx = r["octx"]
        ctxv = (octx[0:M] / octx[M:M + 1]).T      # [4, 64]
        out[core * BL:(core + 1) * BL, 0, 0:P] = hv
        out[core * BL:(core + 1) * BL, 0, P:P + M] = ctxv
    return out


# revision 35
# speedup vs baseline: 1.2186x; 1.0017x over previous
"""Trainium2 Bass kernel for nn_Decoder (attention-LSTM decoder recurrence).

Math (per batch b, T=128 steps, M=P=64):
    repeat t = 0..T-2:
        e = tanh(H @ U_d.T + W_d @ [h; c])          (T, M)
        s = exp(v_d . e)                            (T,)   softmax numerator
        num = sum_t s_t * (H w~[1:] + w~b + w~0 dec_t)_t
        den = sum_t s_t
        y~  = num / den                             (dec folded into num)
        LSTM(y~, h, c) -> h, c                      (i,f,g,o gates)
    final: attend once more; out = [h, ctx]

Sharding: data-parallel over batch. B=32 over 8 cores -> 4 batches/core.

The recurrence is latency-bound (fixed per-instruction engine latencies),
so the step loop is structured to minimize serial engine stages:
  - e_pre = UH + W q is accumulated in PSUM by matmuls (UH re-materialized
    each step from f16 H, off the critical path; q contribution uses
    stride-0 broadcast rhs), so tanh(e) is ONE bias-free ACT instr.
  - hw_all = H w~[1:] + w~b + w~0 dec[b,t] is host-precomputed, folding
    the y~ add into the num matmul (y~ = num/den, one DVE divide).
  - den matmul issues before the num matmuls.
  - gates = Whh [2h; 1] matmuls issue early (hidden behind attention);
    only the rank-1 wih (x) y~ accumulation is on the critical path.
  - sV = (ti+1)*tg runs on gpsimd in parallel with DVE's sU.
State stores 2h / 2c (sigmoid(z) = 0.5 tanh(0.5 z) + 0.5 folding; the 0.5s
live in the host-packed weights).
"""

import numpy as np

B, T, M, P = 32, 128, 64, 64
NCORES = 8
BL = B // NCORES          # batches per core = 4
NG = 2                    # attention groups per core
GB = BL // NG             # batches per attention group = 2

_STATE = {}
NSTEPS = T - 1
DEBUG = False
PSTATE_PAD = False    # PE p-state padding: measured no clock ramp on HW


def _build_nc():
    import contextlib

    import concourse.bacc as bacc
    import concourse.tile as tile
    from concourse import mybir

    f32 = mybir.dt.float32
    f32r = mybir.dt.float32r
    f16 = mybir.dt.float16
    AF = mybir.ActivationFunctionType
    OP = mybir.AluOpType

    nc = bacc.Bacc()

    # ---- per-core sharded data ----
    h_l = nc.declare_dram_parameter("h_l", [BL, T, M], f32, isOutput=False)
    htp = nc.declare_dram_parameter("htp", [GB, M, NG * T], f16, isOutput=False)
    hw_all = nc.declare_dram_parameter("hw_all", [T, 4 * T], f16, isOutput=False)
    st0 = nc.declare_dram_parameter("st0", [2 * P, BL], f16, isOutput=False)
    # ---- replicated packed weights ----
    wd2 = nc.declare_dram_parameter("wd2", [P, 2 * M], f16, isOutput=False)
    udT16 = nc.declare_dram_parameter("udT16", [M, M], f16, isOutput=False)
    v2 = nc.declare_dram_parameter("v2", [2 * M, GB], f16, isOutput=False)
    whbi = nc.declare_dram_parameter("whbi", [P + 2, 4 * P], f16, isOutput=False)
    # ---- outputs ----
    oh = nc.declare_dram_parameter("oh", [P, BL], f32, isOutput=True)
    octx = nc.declare_dram_parameter("octx", [M + 1, BL], f32, isOutput=True)
    if DEBUG:
        dbg_g = nc.declare_dram_parameter("dbg_g", [P, 4 * BL], f32, isOutput=True)

    with tile.TileContext(nc) as tc:
        with contextlib.ExitStack() as ctx:
            consts = ctx.enter_context(tc.tile_pool(name="consts", bufs=1))
            state = ctx.enter_context(tc.tile_pool(name="state", bufs=1))
            loop_sb = ctx.enter_context(tc.tile_pool(name="loop_sb", bufs=3))
            loop_ps = ctx.enter_context(
                tc.tile_pool(name="loop_ps", bufs=1, space="PSUM")
            )
            ep_pool = ctx.enter_context(
                tc.tile_pool(name="ep_ps", bufs=2, space="PSUM")
            )

            # -------- preamble: constants (DMA order = first-use order;
            # haug is only read by the final context matmuls -> last) -----
            sy2 = state.tile([P + 2, BL], f16, tag="SY")
            nc.sync.dma_start(out=sy2[0:P, :], in_=st0[0:P, :])
            cs = state.tile([P, BL], f16, tag="CS")
            nc.sync.dma_start(out=cs, in_=st0[P:2 * P, :])
            wd2_sb = consts.tile([P, 2 * M], f16)
            nc.sync.dma_start(out=wd2_sb, in_=wd2[:])
            udT_sb = consts.tile([M, M], f16)
            nc.sync.dma_start(out=udT_sb, in_=udT16[:])
            htp_sb = []
            for j in range(GB):
                t_ = consts.tile([M, NG * T], f16, tag=f"HTP{j}")
                nc.sync.dma_start(out=t_, in_=htp[j])
                htp_sb.append(t_)
            v2_sb = consts.tile([2 * M, GB], f16)
            nc.sync.dma_start(out=v2_sb, in_=v2[:])
            whbi_sb = consts.tile([P + 2, 4 * P], f16)
            nc.sync.dma_start(out=whbi_sb, in_=whbi[:])
            hw_sb = consts.tile([T, 4 * T], f16)
            nc.sync.dma_start(out=hw_sb, in_=hw_all[:])
            haug = []
            for b in range(BL):
                hb = consts.tile([T, M], f32r, tag=f"HAUG{b}")
                nc.sync.dma_start(out=hb, in_=h_l[b].bitcast(f32r))
                haug.append(hb)
            ones_f = consts.tile([T, 1], f32)
            nc.vector.memset(ones_f, 1.0)
            ones16 = consts.tile([T, 1], f16)
            nc.vector.tensor_copy(out=ones16, in_=ones_f)

            # sy2 = [2h (0:64); y~ (64); 1 (65)], cs = 2c
            ones_g = state.tile([P + 2, BL], f32, tag="ONESG")
            nc.vector.memset(ones_g[P:P + 2, :], 1.0)
            nc.vector.tensor_copy(out=sy2[P:P + 2, :], in_=ones_g[P:P + 2, :])

            # scratch bank for p-state padding matmuls (write-only)
            if PSTATE_PAD:
                pad_ps = ctx.enter_context(
                    tc.tile_pool(name="pad_ps", bufs=1, space="PSUM"))

            def pad(n):
                # dummy matmuls with no waits: they fill PE idle windows so
                # the tensor engine stays busy and ramps to full clock
                if not PSTATE_PAD:
                    return
                for _ in range(n):
                    dtile = pad_ps.tile([M, NG * T], f32, tag="PAD")
                    nc.tensor.matmul(dtile, udT_sb, htp_sb[0],
                                     start=True, stop=True)

            # ---------- one step's attention front: e_pre..num/den ----------
            def attention(t, need_num=True):
                ep = ep_pool.tile([2 * M, NG * T], f32, tag="EP")
                epr = ep.rearrange("p (g t) -> p g t", g=NG)
                # UH accumulation (consts only -> runs during prev LSTM tail)
                for j in range(GB):
                    nc.tensor.matmul(
                        ep[j * M:(j + 1) * M, :], udT_sb, htp_sb[j],
                        start=True, stop=False)
                # qW c-half then h-half, broadcast over t
                csr = cs.rearrange("p (g j) -> p g j", j=GB)
                for j in range(GB):
                    nc.tensor.matmul(
                        epr[j * M:(j + 1) * M, :, :], wd2_sb[:, 0:M],
                        csr[:, :, j].to_broadcast([P, NG, T]),
                        start=False, stop=False, skip_group_check=True)
                syr = sy2[0:P, :].rearrange("p (g j) -> p g j", j=GB)
                for j in range(GB):
                    nc.tensor.matmul(
                        epr[j * M:(j + 1) * M, :, :], wd2_sb[:, M:2 * M],
                        syr[:, :, j].to_broadcast([P, NG, T]),
                        start=False, stop=True, skip_group_check=True)
                # e = tanh(e_pre): single bias-free ACT instr
                e_sb = loop_sb.tile([2 * M, NG * T], f16, tag="e")
                nc.scalar.activation(out=e_sb, in_=ep, func=AF.Tanh)
                # logits
                lg = loop_ps.tile([T, 2 * GB], f32, tag="LG")
                for g in range(NG):
                    nc.tensor.matmul(
                        lg[:, g * GB:(g + 1) * GB],
                        e_sb[:, g * T:(g + 1) * T], v2_sb,
                        start=True, stop=True)
                # softmax numerators
                s_sb = loop_sb.tile([T, BL], f16, tag="s")
                nc.scalar.activation(out=s_sb, in_=lg, func=AF.Exp)
                # den first (its reciprocal overlaps the num matmuls);
                # scalar row-world lives at partition 64 (aligns with the
                # y~ slot in sy2 for the DVE ops)
                dn = loop_ps.tile([P + 2, 2 * BL], f32, tag="DN")
                nc.tensor.matmul(dn[P:P + 1, 0:BL], ones16, s_sb,
                                 start=True, stop=True)
                if need_num:
                    for b in range(BL):
                        nc.tensor.matmul(
                            dn[P:P + 1, BL + b:BL + b + 1],
                            hw_sb[:, 4 * t + b:4 * t + b + 1],
                            s_sb[:, b:b + 1], start=True, stop=True)
                pad(1)
                return dn, s_sb

            # ---------------- main recurrence ----------------
            for t in range(NSTEPS):
                dn, _ = attention(t)
                # y~ = num * (1/den); recip issues right after the den
                # matmul and overlaps the num matmuls; y~ lands in sy2
                rden = loop_sb.tile([P + 2, BL], f32, tag="rden")
                nc.vector.reciprocal(
                    out=rden[P:P + 1, :], in_=dn[P:P + 1, 0:BL])
                nc.vector.tensor_tensor(
                    out=sy2[P:P + 1, :], in0=dn[P:P + 1, BL:2 * BL],
                    in1=rden[P:P + 1, :], op=OP.mult)
                # gates: single complete group, lhsT = [Whh; wih; b]
                gps = loop_ps.tile([P, 4 * BL], f32, tag="G")
                for k in range(4):
                    nc.tensor.matmul(
                        gps[:, k * BL:(k + 1) * BL],
                        whbi_sb[:, k * P:(k + 1) * P], sy2,
                        start=True, stop=True)
                tg = loop_sb.tile([P, 4 * BL], f32, tag="T")
                nc.scalar.activation(out=tg, in_=gps, func=AF.Tanh)
                if DEBUG and t == 0:
                    g32 = loop_sb.tile([P, 4 * BL], f32, tag="g32")
                    nc.vector.tensor_copy(out=g32, in_=gps)
                    nc.sync.dma_start(out=dbg_g[:], in_=g32)
                t_i, t_f = tg[:, 0:BL], tg[:, BL:2 * BL]
                t_o, t_g = tg[:, 2 * BL:3 * BL], tg[:, 3 * BL:4 * BL]
                # CS holds Xc = 2c; X = 0.5*(tf+1)*Xc + (ti+1)*tg = 2c_new
                sU = loop_sb.tile([P, BL], f32, tag="sU")
                nc.vector.scalar_tensor_tensor(
                    out=sU, in0=t_f, scalar=1.0, in1=cs,
                    op0=OP.add, op1=OP.mult)
                sV = loop_sb.tile([P, BL], f32, tag="sV")
                nc.vector.scalar_tensor_tensor(
                    out=sV, in0=t_i, scalar=1.0, in1=t_g,
                    op0=OP.add, op1=OP.mult)
                nc.vector.scalar_tensor_tensor(
                    out=cs, in0=sU, in1=sV, scalar=0.5,
                    op0=OP.mult, op1=OP.add)
                th = loop_sb.tile([P, BL], f32, tag="th")
                nc.scalar.activation(out=th, in_=cs, func=AF.Tanh, scale=0.5)
                # 2h_new = (to+1)*th
                nc.vector.scalar_tensor_tensor(
                    out=sy2[0:P, :], in0=t_o, scalar=1.0, in1=th,
                    op0=OP.add, op1=OP.mult)

            # ---------------- final attend + outputs ----------------
            dn, s_fin = attention(None, need_num=False)
            s_fr = loop_sb.tile([T, BL], f32r, tag="sfr")
            nc.vector.tensor_copy(out=s_fr, in_=s_fin)
            ctx_ps = loop_ps.tile([M, 2 * BL], f32, tag="CTXF")
            for b in range(BL):
                nc.tensor.matmul(
                    ctx_ps[:, 2 * b:2 * b + 2], haug[b],
                    s_fr[:, b:b + 1].to_broadcast([T, 2]),
                    start=True, stop=True)
            ctx_out = loop_sb.tile([M, BL], f32, tag="ctxout")
            nc.vector.tensor_copy(
                out=ctx_out,
                in_=ctx_ps.rearrange("p (b two) -> p b two", two=2)[:, :, 0])
            den_out = loop_sb.tile([P + 2, BL], f32, tag="denout")
            nc.vector.tensor_copy(
                out=den_out[P:P + 1, :], in_=dn[P:P + 1, 0:BL])
            oh_sb = loop_sb.tile([P, BL], f32, tag="ohsb")
            nc.vector.tensor_copy(out=oh_sb, in_=sy2[0:P, :])
            nc.sync.dma_start(out=octx[0:M, :], in_=ctx_out)
            nc.sync.dma_start(out=octx[M:M + 1, :], in_=den_out[P:P + 1, :])
            nc.gpsimd.dma_start(out=oh[:], in_=oh_sb)

    nc.finalize()
    return nc


def _pack_weights(W_d, U_d, v_d, w_tilde_W, w_tilde_b, W_ih, W_hh, b_ih, b_hh):
    f16 = np.float16
    # q = [h;c] stored as 2h;2c -> fold 0.5 into W_d^T halves
    # wd2 cols 0:M = c-half lhsT, cols M:2M = h-half lhsT
    wd2 = np.zeros((P, 2 * M), dtype=f16)
    wd2[:, 0:M] = 0.5 * W_d[:, P:2 * P].T
    wd2[:, M:2 * M] = 0.5 * W_d[:, 0:P].T
    udT16 = np.ascontiguousarray(U_d.T, dtype=f16)
    v2 = np.zeros((2 * M, GB), dtype=f16)
    v2[0:M, 0] = v_d[0]
    v2[M:2 * M, 1] = v_d[0]
    bsum = (b_ih + b_hh).astype(np.float32)
    wih = W_ih[:, 0].astype(np.float32)
    # torch gate order i,f,g,o; our column order i,f,o,g.
    # sigmoid gates (i,f,o): pre-scale 0.5 (sigmoid(z) = 0.5 tanh(0.5 z)+0.5)
    # h input is 2h -> extra 0.5 on W_hh blocks.
    src = [0, 1, 3, 2]                    # i, f, o, g row-blocks in torch order
    sig = [0.5, 0.5, 0.5, 1.0]
    whbi = np.zeros((P + 2, 4 * P), dtype=f16)
    for k in range(4):
        blk = slice(src[k] * P, (src[k] + 1) * P)
        whbi[0:P, k * P:(k + 1) * P] = sig[k] * 0.5 * W_hh[blk].T
        whbi[P, k * P:(k + 1) * P] = sig[k] * wih[blk]
        whbi[P + 1, k * P:(k + 1) * P] = sig[k] * bsum[blk]
    return dict(wd2=wd2, udT16=udT16, v2=v2, whbi=whbi)


def kernel(H, dec_data, d_1, s_1, W_d, U_d, v_d, w_tilde_W, w_tilde_b,
           W_ih, W_hh, b_ih, b_hh, T=None):
    from concourse.bass_utils import run_bass_kernel_spmd

    H = np.asarray(H, dtype=np.float32)
    dec_data = np.asarray(dec_data, dtype=np.float32)
    d_1 = np.asarray(d_1, dtype=np.float32)
    s_1 = np.asarray(s_1, dtype=np.float32)
    W_d = np.asarray(W_d, np.float32)
    w_tilde_W = np.asarray(w_tilde_W, np.float32)
    w_tilde_b = np.asarray(w_tilde_b, np.float32)

    if "nc" not in _STATE:
        _STATE["nc"] = _build_nc()
    nc = _STATE["nc"]

    wpack = _pack_weights(
        W_d, np.asarray(U_d, np.float32),
        np.asarray(v_d, np.float32), w_tilde_W, w_tilde_b,
        np.asarray(W_ih, np.float32), np.asarray(W_hh, np.float32),
        np.asarray(b_ih, np.float32), np.asarray(b_hh, np.float32),
    )

    wt1 = w_tilde_W[0, 1:M + 1]                         # [64]
    w0 = w_tilde_W[0, 0]
    wtb = w_tilde_b[0]

    in_maps = []
    for core in range(NCORES):
        sl = slice(core * BL, (core + 1) * BL)
        h_c = H[sl]                                     # [4, T, M]
        # htp[j][m, g*T + t] = H[2g+j][t, m]
        htp = np.zeros((GB, M, NG * 128), dtype=np.float16)
        for j in range(GB):
            for g in range(NG):
                htp[j, :, g * 128:(g + 1) * 128] = h_c[NG * g + j].T
        # hw_all[t, 4*s + b] = (H_b @ wt1)[t] + wtb + w0*dec[b, s]
        hwb = h_c @ wt1 + wtb                           # [4, T]
        dec_c = dec_data[sl, :, 0]                      # [4, T]
        # X[s, b, t] = hwb[b, t] + w0*dec[b, s]; want hw_all[t, 4*s+b]
        hw_all = (hwb[None, :, :] + (w0 * dec_c).T[:, :, None]
                  ).transpose(2, 0, 1).reshape(128, 4 * 128)
        st = np.concatenate(
            [2.0 * d_1[0, sl].T, 2.0 * s_1[0, sl].T], axis=0
        ).astype(np.float16)                            # [128, 4]
        m = dict(wpack)
        m.update(
            h_l=np.ascontiguousarray(h_c),
            htp=htp,
            hw_all=np.ascontiguousarray(hw_all.astype(np.float16)),
            st0=np.ascontiguousarray(st),
        )
        in_maps.append(m)

    res = run_bass_kernel_spmd(nc, in_maps, list(range(NCORES)))
    _STATE["last_results"] = res

    out = np.zeros((B, 1, P + M), dtype=np.float32)
    for core in range(NCORES):
        r = res.results[core]
        hv = r["oh"].T * 0.5                      # [4, 64]  (state was 2h)
        octx = r["octx"]
        ctxv = (octx[0:M] / octx[M:M + 1]).T      # [4, 64]
        out[core * BL:(core + 1) * BL, 0, 0:P] = hv
        out[core * BL:(core + 1) * BL, 0, P:P + M] = ctxv
    return out
